# revision 17
# baseline (speedup 1.0000x reference)
"""Trainium2 Bass kernel for nn_DecoderTASA (GRU decoder step + log-softmax).

Strategy (8 NeuronCores, SPMD, no collectives):
  - w_lin/b_lin sharded over vocab (6400 rows/core, padded 50257 -> 51200).
  - GRU weights replicated on every core (H=1024 is small); each core
    computes the identical h_new on the tensor engine, then its own
    1/8 slice of the logits as a tensor-parallel GEMV.
  - All matmul weights are pre-transposed on host into the PE's lhsT
    block layout and cast to bf16 (memory-bound problem: halves HBM
    traffic; logits error ~1e-3 absolute).
  - Each core also reduces its logits slice to per-partition
    (max, sum-of-exp) stats; the host combines the 8x128 stat pairs
    into the global log-sum-exp and subtracts it while un-sharding.

Outputs per core: logits slice [128, 50] (p-major), stats [128, 2],
h_new [128, 8]. Host reassembles (out [1, V], h_new [1, 1, H]).
"""

import sys

if '/opt/trn_rl_repo' not in sys.path:
    sys.path.insert(0, '/opt/trn_rl_repo')

import numpy as np
import ml_dtypes

from concourse import mybir, bacc, tile
from concourse import bass_utils

BF16 = ml_dtypes.bfloat16
F8 = ml_dtypes.float8_e4m3

H = 1024
V = 50257
N_CORES = 8
VP_CORE = 6400            # padded vocab rows per core
V_PAD = N_CORES * VP_CORE  # 51200
T_CORE = VP_CORE // 128    # 50 v-tiles per core
KH = H // 128              # 8 h-chunks
NEG_BIG = -1.0e30
WL_SCALE = 16.0
X_SCALE = 64.0

_PROGRAM_CACHE = {}


def _build_program():
    """Build + compile the SPMD Bass program (input-value independent)."""
    f32 = mybir.dt.float32
    bf16 = mybir.dt.bfloat16

    nc = bacc.Bacc("TRN2", target_bir_lowering=False, debug=False,
                   enable_asserts=False, num_devices=N_CORES)

    # ---- DRAM I/O ----
    f8 = mybir.dt.float8e4
    wl = nc.dram_tensor("wl", [128, T_CORE * KH * 128], f8, kind="ExternalInput")
    wi = nc.dram_tensor("wi", [128, 24 * KH * 128], f8, kind="ExternalInput")
    wh = nc.dram_tensor("wh", [128, 24 * KH * 128], bf16, kind="ExternalInput")
    # all small inputs packed into one tensor (single DMA):
    # cols 0:8 x | 8:16 h | 16:40 b_ih | 40:64 b_hh | 64:114 b_lin | 114 th | 115 mu | 116 tau
    PK_W = 2 * KH + 48 + T_CORE + 3
    pk = nc.dram_tensor("pk", [128, PK_W], f32, kind="ExternalInput")

    logits_out = nc.dram_tensor("logits_out", [128, T_CORE], f32, kind="ExternalOutput")
    stats_out = nc.dram_tensor("stats_out", [128, 2], f32, kind="ExternalOutput")
    h_out = nc.dram_tensor("h_out", [128, KH], f32, kind="ExternalOutput")

    AF = mybir.ActivationFunctionType

    with tile.TileContext(nc) as tc:
        with tc.tile_pool(name="gru_w", bufs=1) as gru_w, \
             tc.tile_pool(name="wl_pool", bufs=5) as wl_pool, \
             tc.tile_pool(name="small", bufs=1) as small, \
             tc.tile_pool(name="psum_g", bufs=1, space="PSUM") as psum_g, \
             tc.tile_pool(name="psum_l", bufs=4, space="PSUM") as psum_pool:

            # --- all small inputs in ONE DMA on the ACT HWDGE ring, so the
            # big weight streams on the SP ring start immediately ---
            pk_sb = small.tile([128, PK_W], f32, tag="pk")
            nc.scalar.dma_start(pk_sb[:], pk[:])
            x_sb = pk_sb[:, 0:KH]
            h_sb = pk_sb[:, KH:2 * KH]
            bi_sb = pk_sb[:, 16:40]
            bh_sb = pk_sb[:, 40:64]
            bl_sb = pk_sb[:, 64:64 + T_CORE]
            th_sb = pk_sb[:, 114:115]
            mu_sb = pk_sb[:, 115:116]
            ta_sb = pk_sb[:, 116:117]

            # --- GRU weights: 4 sub-DMAs each so matmuls start early ---
            wi_sb = gru_w.tile([128, 24 * KH * 128], f8, tag="wi")
            wh_sb = gru_w.tile([128, 24 * KH * 128], bf16, tag="wh")
            GRU_SUB = 6 * KH * 128  # 6 (g,j) units per sub-DMA
            # wh (bf16, 6.3MB) first; wi is fp8 and arrives fast after
            for s in range(4):
                sl = slice(s * GRU_SUB, (s + 1) * GRU_SUB)
                nc.sync.dma_start(wh_sb[:, sl], wh[:, sl])
            for s in range(4):
                sl = slice(s * GRU_SUB, (s + 1) * GRU_SUB)
                nc.sync.dma_start(wi_sb[:, sl], wi[:, sl])

            # --- w_lin^T stream: 5 chunks of 10 v-tiles (2.62 MB each) ---
            WL_CHUNK_T = 10
            WL_CHUNK = WL_CHUNK_T * KH * 128
            wl_tiles = []
            for s in range(T_CORE // WL_CHUNK_T):
                wt = wl_pool.tile([128, WL_CHUNK], f8, tag="wl")
                sl = slice(s * WL_CHUNK, (s + 1) * WL_CHUNK)
                nc.sync.dma_start(wt[:], wl[:, sl])
                wl_tiles.append(wt)

            # --- prob = sigmoid(theta + mu*tau); x_bf = emb_row * prob ---
            scr = small.tile([128, 1], f32, tag="scr")
            nc.vector.tensor_mul(scr[:], mu_sb[:], ta_sb[:])
            nc.vector.tensor_add(scr[:], scr[:], th_sb[:])
            prob = small.tile([128, 1], f32, tag="prob")
            nc.scalar.activation(prob[:], scr[:], AF.Sigmoid)

            prob64 = small.tile([128, 1], f32, tag="prob64")
            nc.vector.tensor_scalar_mul(prob64[:], prob[:], float(X_SCALE))
            x_bf = small.tile([128, KH], f8, tag="xbf")
            nc.scalar.activation(x_bf[:], x_sb[:], AF.Copy, scale=prob64[:, 0:1])
            h_bf = small.tile([128, KH], bf16, tag="hbf")
            nc.vector.tensor_copy(h_bf[:], h_sb[:])

            # --- GRU gate GEMVs on PE: gi = W_ih @ x, gh = W_hh @ h ---
            # columns of gi/gh psum: col = g*8 + j  (g in r,z,n; j = m-chunk)
            gi_ps = psum_g.tile([128, 24], f32, tag="gi")
            gh_ps = psum_g.tile([128, 24], f32, tag="gh")
            for col in range(24):
                for k in range(KH):
                    blk = slice((col * KH + k) * 128, (col * KH + k + 1) * 128)
                    nc.tensor.matmul(gi_ps[:, col:col + 1], wi_sb[:, blk],
                                     x_bf[:, k:k + 1],
                                     start=(k == 0), stop=(k == KH - 1))
            for col in range(24):
                for k in range(KH):
                    blk = slice((col * KH + k) * 128, (col * KH + k + 1) * 128)
                    nc.tensor.matmul(gh_ps[:, col:col + 1], wh_sb[:, blk],
                                     h_bf[:, k:k + 1],
                                     start=(k == 0), stop=(k == KH - 1))

            # --- gates ---
            gsum_i = small.tile([128, 24], f32, tag="gsi")
            gsum_h = small.tile([128, 24], f32, tag="gsh")
            nc.vector.scalar_tensor_tensor(
                gsum_i[:], gi_ps[:], 1.0 / (WL_SCALE * X_SCALE), bi_sb[:],
                op0=mybir.AluOpType.mult, op1=mybir.AluOpType.add)
            nc.vector.tensor_add(gsum_h[:], gh_ps[:], bh_sb[:])

            rz = small.tile([128, 16], f32, tag="rz")
            nc.vector.tensor_add(rz[:], gsum_i[:, 0:16], gsum_h[:, 0:16])
            nc.scalar.activation(rz[:], rz[:], AF.Sigmoid)  # r | z

            n_t = small.tile([128, KH], f32, tag="nt")
            nc.vector.tensor_mul(n_t[:], rz[:, 0:8], gsum_h[:, 16:24])
            nc.vector.tensor_add(n_t[:], n_t[:], gsum_i[:, 16:24])
            nc.scalar.activation(n_t[:], n_t[:], AF.Tanh)

            # h_new = n + z * (h_old - n)
            hn = small.tile([128, KH], f32, tag="hn")
            nc.vector.tensor_sub(hn[:], h_sb[:], n_t[:])
            nc.vector.tensor_mul(hn[:], hn[:], rz[:, 8:16])
            nc.vector.tensor_add(hn[:], hn[:], n_t[:])
            nc.scalar.dma_start(h_out[:], hn[:])

            # fp8 residual split of h_new: rhs2[:, 2k] = hi_k, [:, 2k+1] = lo_k
            rhs2 = small.tile([128, 2 * KH], f8, tag="rhs2")
            nc.vector.tensor_copy(rhs2[:, 0:2 * KH:2], hn[:])
            hi_f32 = small.tile([128, KH], f32, tag="hif")
            nc.vector.tensor_copy(hi_f32[:], rhs2[:, 0:2 * KH:2])
            nc.vector.tensor_sub(rhs2[:, 1:2 * KH:2], hn[:], hi_f32[:])

            # --- big GEMV: logits slice, 50 v-tiles x 8 k-chunks ---
            logits_sb = small.tile([128, T_CORE], f32, tag="lg")
            NG = (T_CORE + 7) // 8
            gmax = small.tile([128, NG], f32, tag="gmax")
            gsum = small.tile([128, NG], f32, tag="gsum")
            e_scr = small.tile([128, 8], f32, tag="escr")
            negm = small.tile([128, 1], f32, tag="negm")

            def mm_group(gidx, t0, nt):
                # N=2 matmuls: psum cols (2tt, 2tt+1) get hi/lo partials
                lp = psum_pool.tile([128, 2 * nt], f32, tag="lp")
                for tt in range(nt):
                    t = t0 + tt
                    chunk, t_in = divmod(t, WL_CHUNK_T)
                    wt = wl_tiles[chunk]
                    for k in range(KH):
                        blk = slice((t_in * KH + k) * 128, (t_in * KH + k + 1) * 128)
                        nc.tensor.matmul(lp[:, 2 * tt:2 * tt + 2], wt[:, blk],
                                         rhs2[:, 2 * k:2 * k + 2],
                                         start=(k == 0), stop=(k == KH - 1))
                # scaled copy PSUM->SBUF, hi+lo combine, bias, running stats
                sl = slice(t0, t0 + nt)
                tmp = small.tile([128, 16], f32, tag="lgtmp")
                nc.scalar.mul(tmp[:, 0:2 * nt], lp[:], 1.0 / WL_SCALE)
                nc.vector.tensor_add(logits_sb[:, sl],
                                     tmp[:, 0:2 * nt:2], tmp[:, 1:2 * nt:2])
                nc.vector.tensor_add(logits_sb[:, sl], logits_sb[:, sl],
                                     bl_sb[:, sl])
                nc.vector.reduce_max(gmax[:, gidx:gidx + 1], logits_sb[:, sl],
                                     axis=mybir.AxisListType.X)
                nc.vector.tensor_scalar_mul(negm[:], gmax[:, gidx:gidx + 1], -1.0)
                nc.scalar.activation(e_scr[:, 0:nt], logits_sb[:, sl], AF.Exp,
                                     bias=negm[:, 0:1],
                                     accum_out=gsum[:, gidx:gidx + 1])

            for tg in range(T_CORE // 8):
                mm_group(tg, tg * 8, 8)
            rem = T_CORE - (T_CORE // 8) * 8
            if rem:
                mm_group(NG - 1, T_CORE - rem, rem)

            # --- combine per-group stats into (m, s) ---
            nc.scalar.dma_start(logits_out[:], logits_sb[:])
            stats = small.tile([128, 2], f32, tag="st")
            nc.vector.reduce_max(stats[:, 0:1], gmax[:],
                                 axis=mybir.AxisListType.X)
            m_neg = small.tile([128, 1], f32, tag="mn")
            nc.vector.tensor_scalar_mul(m_neg[:], stats[:, 0:1], -1.0)
            eg = small.tile([128, NG], f32, tag="eg")
            nc.scalar.activation(eg[:], gmax[:], AF.Exp, bias=m_neg[:, 0:1])
            nc.vector.tensor_mul(eg[:], eg[:], gsum[:])
            nc.vector.reduce_sum(stats[:, 1:2], eg[:],
                                 axis=mybir.AxisListType.X)
            nc.scalar.dma_start(stats_out[:], stats[:])

    nc.compile()
    return nc


def _prep_inputs(inputs, hidden, tau, emb, w_ih, w_hh, b_ih, b_hh,
                 w_lin, b_lin, theta, mu):
    """Host-side sharding: pad/permute/cast into the device layouts."""
    ix = int(np.asarray(inputs).reshape(-1)[0])

    # w_lin^T shards: [c, p, t, k, m] <- w_lin[c*6400 + t*128 + m, k*128 + p]
    wl_pad = np.zeros((V_PAD, H), dtype=F8)
    wl_pad[:V] = (np.asarray(w_lin, dtype=np.float32) * WL_SCALE).astype(F8)
    A = wl_pad.reshape(N_CORES, T_CORE, 128, KH, 128)       # (c, t, m, k, p)
    WL = np.ascontiguousarray(A.transpose(0, 4, 1, 3, 2))   # (c, p, t, k, m)
    WL = WL.reshape(N_CORES, 128, T_CORE * KH * 128)

    # GRU weights: [p, g, j, k, m] <- w[g*1024 + j*128 + m, k*128 + p]
    def gru_t(w, dt, scale=1.0):
        B = (np.asarray(w, dtype=np.float32) * scale).astype(dt)
        B = B.reshape(3, KH, 128, KH, 128)                  # (g, j, m, k, p)
        return np.ascontiguousarray(B.transpose(4, 0, 1, 3, 2)).reshape(128, -1)

    WI = gru_t(w_ih, F8, WL_SCALE)
    WH = gru_t(w_hh, BF16)

    def gbias(b):
        return np.ascontiguousarray(
            np.asarray(b, dtype=np.float32).reshape(3, KH, 128).transpose(2, 0, 1)
        ).reshape(128, 24)

    BI = gbias(b_ih)
    BH = gbias(b_hh)

    bl_pad = np.full(V_PAD, NEG_BIG, dtype=np.float32)
    bl_pad[:V] = np.asarray(b_lin, dtype=np.float32)
    BL = np.ascontiguousarray(
        bl_pad.reshape(N_CORES, T_CORE, 128).transpose(0, 2, 1))  # (c, p, t)

    x_row = np.asarray(emb[ix], dtype=np.float32)
    X = np.ascontiguousarray(x_row.reshape(KH, 128).T)      # [p, k]
    Hh = np.ascontiguousarray(
        np.asarray(hidden, dtype=np.float32).reshape(KH, 128).T)

    # packed small-input tensor, layout must match _build_program
    PK_W = 2 * KH + 48 + T_CORE + 3
    PK = np.empty((N_CORES, 128, PK_W), dtype=np.float32)
    PK[:, :, 0:KH] = X
    PK[:, :, KH:2 * KH] = Hh
    PK[:, :, 16:40] = BI
    PK[:, :, 40:64] = BH
    PK[:, :, 64:64 + T_CORE] = BL
    PK[:, :, 114] = float(np.asarray(theta)[ix])
    PK[:, :, 115] = float(np.asarray(mu)[ix])
    PK[:, :, 116] = float(np.asarray(tau))

    in_maps = []
    for c in range(N_CORES):
        in_maps.append({
            "wl": WL[c], "wi": WI, "wh": WH, "pk": PK[c],
        })
    return in_maps


def kernel(inputs, hidden, tau, emb, w_ih, w_hh, b_ih, b_hh,
           w_lin, b_lin, theta, mu):
    key = "prog"
    if key not in _PROGRAM_CACHE:
        _PROGRAM_CACHE[key] = _build_program()
    nc = _PROGRAM_CACHE[key]

    in_maps = _prep_inputs(inputs, hidden, tau, emb, w_ih, w_hh, b_ih, b_hh,
                           w_lin, b_lin, theta, mu)

    res = bass_utils.run_bass_kernel_spmd(nc, in_maps,
                                          core_ids=list(range(N_CORES)))

    # ---- host unshard ----
    logits_big = np.concatenate(
        [res.results[c]["logits_out"] for c in range(N_CORES)], axis=1)
    # column order: c*T_CORE + t ; logit[v] with v = (c*T_CORE + t)*128 + p
    logits_full = logits_big.T.reshape(V_PAD)[:V]

    m1 = np.concatenate(
        [res.results[c]["stats_out"][:, 0] for c in range(N_CORES)])
    s1 = np.concatenate(
        [res.results[c]["stats_out"][:, 1] for c in range(N_CORES)])
    m1 = m1.astype(np.float64)
    s1 = s1.astype(np.float64)
    M = m1.max()
    S = float((np.exp(m1 - M) * s1).sum())
    lse = M + np.log(S)

    out = (logits_full - np.float32(lse))[None, :].astype(np.float32)

    h_new = res.results[0]["h_out"].T.reshape(1, 1, H).astype(np.float32)
    return out, h_new


# revision 18
# speedup vs baseline: 1.1239x; 1.1239x over previous
"""Trainium2 Bass kernel for nn_DecoderTASA (GRU decoder step + log-softmax).

Strategy (8 NeuronCores, SPMD, no collectives — they cost ~80us under
this runtime, measured, so every core runs independently):
  - w_lin/b_lin sharded over vocab (6400 rows/core, padded 50257 -> 51200);
    tensor-parallel GEMV on the PE, one 128x128 lhsT block per (v-tile, k).
  - GRU weights replicated on every core (H=1024 is small, per the
    sharding hint); each core computes the identical h_new, then its own
    1/8 logits slice.
  - Precision (memory-bound, so bytes == time): w_lin and w_ih in
    fp8 e4m3 scaled x16 (w_ih feeds the tiny `x` path, error negligible);
    w_hh stays bf16 to protect h_new. h_new enters the big GEMV as an
    fp8 (hi, lo) residual pair accumulated into the same psum column,
    which keeps the fp8-w_lin logits error at the bf16-h level
    (~2.6e-3 scale-relative).
  - Each core reduces its logits slice to per-group running
    (max, sum-of-exp) stats during the GEMV phase; the host combines the
    8x128 stat pairs into the global log-sum-exp and subtracts it while
    un-sharding (the only cross-core step; collectives are unusable).

Outputs per core: logits slice [128, 50] (p-major), stats [128, 2],
h_new [128, 8]. Host reassembles (out [1, V], h_new [1, 1, H]).
Measured: ~66us HW exec per core (vs ~600us naive single-core f32);
~46us of that is the 16MB/core HBM stream at ~330GB/s + ~17us fixed
Tile preamble/epilogue.
"""

import sys

if '/opt/trn_rl_repo' not in sys.path:
    sys.path.insert(0, '/opt/trn_rl_repo')

import numpy as np
import ml_dtypes

from concourse import mybir, bacc, tile
from concourse import bass_utils

BF16 = ml_dtypes.bfloat16
F8 = ml_dtypes.float8_e4m3

H = 1024
V = 50257
N_CORES = 8
VP_CORE = 6400            # padded vocab rows per core
V_PAD = N_CORES * VP_CORE  # 51200
T_CORE = VP_CORE // 128    # 50 v-tiles per core
KH = H // 128              # 8 h-chunks
NEG_BIG = -1.0e30
WL_SCALE = 16.0
X_SCALE = 64.0

_PROGRAM_CACHE = {}


def _build_program():
    """Build + compile the SPMD Bass program (input-value independent)."""
    f32 = mybir.dt.float32
    bf16 = mybir.dt.bfloat16

    nc = bacc.Bacc("TRN2", target_bir_lowering=False, debug=False,
                   enable_asserts=False, num_devices=N_CORES)

    # ---- DRAM I/O ----
    f8 = mybir.dt.float8e4
    wl = nc.dram_tensor("wl", [128, T_CORE * KH * 128], f8, kind="ExternalInput")
    wi = nc.dram_tensor("wi", [128, 24 * KH * 128], f8, kind="ExternalInput")
    wh = nc.dram_tensor("wh", [128, 24 * KH * 128], bf16, kind="ExternalInput")
    # all small inputs packed into one tensor (single DMA):
    # cols 0:8 x | 8:16 h | 16:40 b_ih | 40:64 b_hh | 64:114 b_lin | 114 th | 115 mu | 116 tau
    PK_W = 2 * KH + 48 + T_CORE + 3
    pk = nc.dram_tensor("pk", [128, PK_W], f32, kind="ExternalInput")

    logits_out = nc.dram_tensor("logits_out", [128, T_CORE], f32, kind="ExternalOutput")
    stats_out = nc.dram_tensor("stats_out", [128, 2], f32, kind="ExternalOutput")
    h_out = nc.dram_tensor("h_out", [128, KH], f32, kind="ExternalOutput")

    AF = mybir.ActivationFunctionType

    with tile.TileContext(nc) as tc:
        with tc.tile_pool(name="gru_w", bufs=1) as gru_w, \
             tc.tile_pool(name="wl_pool", bufs=5) as wl_pool, \
             tc.tile_pool(name="small", bufs=1) as small, \
             tc.tile_pool(name="psum_g", bufs=1, space="PSUM") as psum_g, \
             tc.tile_pool(name="psum_l", bufs=4, space="PSUM") as psum_pool:

            # --- all small inputs in ONE DMA on the ACT HWDGE ring, so the
            # big weight streams on the SP ring start immediately ---
            pk_sb = small.tile([128, PK_W], f32, tag="pk")
            nc.scalar.dma_start(pk_sb[:], pk[:])
            x_sb = pk_sb[:, 0:KH]
            h_sb = pk_sb[:, KH:2 * KH]
            bi_sb = pk_sb[:, 16:40]
            bh_sb = pk_sb[:, 40:64]
            bl_sb = pk_sb[:, 64:64 + T_CORE]
            th_sb = pk_sb[:, 114:115]
            mu_sb = pk_sb[:, 115:116]
            ta_sb = pk_sb[:, 116:117]

            # --- GRU weights: 4 sub-DMAs each so matmuls start early ---
            wi_sb = gru_w.tile([128, 24 * KH * 128], f8, tag="wi")
            wh_sb = gru_w.tile([128, 24 * KH * 128], bf16, tag="wh")
            GRU_SUB = 6 * KH * 128  # 6 (g,j) units per sub-DMA
            # wh (bf16, 6.3MB) first; wi is fp8 and arrives fast after
            for s in range(4):
                sl = slice(s * GRU_SUB, (s + 1) * GRU_SUB)
                nc.sync.dma_start(wh_sb[:, sl], wh[:, sl])
            for s in range(4):
                sl = slice(s * GRU_SUB, (s + 1) * GRU_SUB)
                nc.sync.dma_start(wi_sb[:, sl], wi[:, sl])

            # --- w_lin^T stream: 5 chunks of 10 v-tiles (2.62 MB each) ---
            WL_CHUNK_T = 10
            WL_CHUNK = WL_CHUNK_T * KH * 128
            wl_tiles = []
            for s in range(T_CORE // WL_CHUNK_T):
                wt = wl_pool.tile([128, WL_CHUNK], f8, tag="wl")
                sl = slice(s * WL_CHUNK, (s + 1) * WL_CHUNK)
                nc.sync.dma_start(wt[:], wl[:, sl])
                wl_tiles.append(wt)

            # --- prob = sigmoid(theta + mu*tau); x_bf = emb_row * prob ---
            scr = small.tile([128, 1], f32, tag="scr")
            nc.vector.tensor_mul(scr[:], mu_sb[:], ta_sb[:])
            nc.vector.tensor_add(scr[:], scr[:], th_sb[:])
            prob = small.tile([128, 1], f32, tag="prob")
            nc.scalar.activation(prob[:], scr[:], AF.Sigmoid)

            prob64 = small.tile([128, 1], f32, tag="prob64")
            nc.vector.tensor_scalar_mul(prob64[:], prob[:], float(X_SCALE))
            x_bf = small.tile([128, KH], f8, tag="xbf")
            nc.scalar.activation(x_bf[:], x_sb[:], AF.Copy, scale=prob64[:, 0:1])
            h_bf = small.tile([128, KH], bf16, tag="hbf")
            nc.vector.tensor_copy(h_bf[:], h_sb[:])

            # --- GRU gate GEMVs on PE: gi = W_ih @ x, gh = W_hh @ h ---
            # columns of gi/gh psum: col = g*8 + j  (g in r,z,n; j = m-chunk)
            gi_ps = psum_g.tile([128, 24], f32, tag="gi")
            gh_ps = psum_g.tile([128, 24], f32, tag="gh")
            for col in range(24):
                for k in range(KH):
                    blk = slice((col * KH + k) * 128, (col * KH + k + 1) * 128)
                    nc.tensor.matmul(gi_ps[:, col:col + 1], wi_sb[:, blk],
                                     x_bf[:, k:k + 1],
                                     start=(k == 0), stop=(k == KH - 1))
            for col in range(24):
                for k in range(KH):
                    blk = slice((col * KH + k) * 128, (col * KH + k + 1) * 128)
                    nc.tensor.matmul(gh_ps[:, col:col + 1], wh_sb[:, blk],
                                     h_bf[:, k:k + 1],
                                     start=(k == 0), stop=(k == KH - 1))

            # --- gates ---
            gsum_i = small.tile([128, 24], f32, tag="gsi")
            gsum_h = small.tile([128, 24], f32, tag="gsh")
            nc.vector.scalar_tensor_tensor(
                gsum_i[:], gi_ps[:], 1.0 / (WL_SCALE * X_SCALE), bi_sb[:],
                op0=mybir.AluOpType.mult, op1=mybir.AluOpType.add)
            nc.vector.tensor_add(gsum_h[:], gh_ps[:], bh_sb[:])

            rz = small.tile([128, 16], f32, tag="rz")
            nc.vector.tensor_add(rz[:], gsum_i[:, 0:16], gsum_h[:, 0:16])
            nc.scalar.activation(rz[:], rz[:], AF.Sigmoid)  # r | z

            n_t = small.tile([128, KH], f32, tag="nt")
            nc.vector.tensor_mul(n_t[:], rz[:, 0:8], gsum_h[:, 16:24])
            nc.vector.tensor_add(n_t[:], n_t[:], gsum_i[:, 16:24])
            nc.scalar.activation(n_t[:], n_t[:], AF.Tanh)

            # h_new = n + z * (h_old - n)
            hn = small.tile([128, KH], f32, tag="hn")
            nc.vector.tensor_sub(hn[:], h_sb[:], n_t[:])
            nc.vector.tensor_mul(hn[:], hn[:], rz[:, 8:16])
            nc.vector.tensor_add(hn[:], hn[:], n_t[:])
            nc.scalar.dma_start(h_out[:], hn[:])

            # fp8 residual split of h_new: rhs2[:, 2k] = hi_k, [:, 2k+1] = lo_k
            rhs2 = small.tile([128, 2 * KH], f8, tag="rhs2")
            nc.vector.tensor_copy(rhs2[:, 0:2 * KH:2], hn[:])
            hi_f32 = small.tile([128, KH], f32, tag="hif")
            nc.vector.tensor_copy(hi_f32[:], rhs2[:, 0:2 * KH:2])
            nc.vector.tensor_sub(rhs2[:, 1:2 * KH:2], hn[:], hi_f32[:])

            # --- big GEMV: logits slice, 50 v-tiles x 8 k-chunks ---
            logits_sb = small.tile([128, T_CORE], f32, tag="lg")
            NG = (T_CORE + 7) // 8
            gmax = small.tile([128, NG], f32, tag="gmax")
            gsum = small.tile([128, NG], f32, tag="gsum")
            e_scr = small.tile([128, 8], f32, tag="escr")
            negm = small.tile([128, 1], f32, tag="negm")

            def mm_group(gidx, t0, nt):
                # N=2 matmuls: psum cols (2tt, 2tt+1) get hi/lo partials
                lp = psum_pool.tile([128, 2 * nt], f32, tag="lp")
                for tt in range(nt):
                    t = t0 + tt
                    chunk, t_in = divmod(t, WL_CHUNK_T)
                    wt = wl_tiles[chunk]
                    for k in range(KH):
                        blk = slice((t_in * KH + k) * 128, (t_in * KH + k + 1) * 128)
                        nc.tensor.matmul(lp[:, 2 * tt:2 * tt + 2], wt[:, blk],
                                         rhs2[:, 2 * k:2 * k + 2],
                                         start=(k == 0), stop=(k == KH - 1))
                # scaled copy PSUM->SBUF, hi+lo combine, bias, running stats
                sl = slice(t0, t0 + nt)
                tmp = small.tile([128, 16], f32, tag="lgtmp")
                nc.scalar.mul(tmp[:, 0:2 * nt], lp[:], 1.0 / WL_SCALE)
                nc.vector.tensor_add(logits_sb[:, sl],
                                     tmp[:, 0:2 * nt:2], tmp[:, 1:2 * nt:2])
                nc.vector.tensor_add(logits_sb[:, sl], logits_sb[:, sl],
                                     bl_sb[:, sl])
                nc.vector.reduce_max(gmax[:, gidx:gidx + 1], logits_sb[:, sl],
                                     axis=mybir.AxisListType.X)
                nc.vector.tensor_scalar_mul(negm[:], gmax[:, gidx:gidx + 1], -1.0)
                nc.scalar.activation(e_scr[:, 0:nt], logits_sb[:, sl], AF.Exp,
                                     bias=negm[:, 0:1],
                                     accum_out=gsum[:, gidx:gidx + 1])

            for tg in range(T_CORE // 8):
                mm_group(tg, tg * 8, 8)
            rem = T_CORE - (T_CORE // 8) * 8
            if rem:
                mm_group(NG - 1, T_CORE - rem, rem)

            # --- combine per-group stats into (m, s) ---
            nc.scalar.dma_start(logits_out[:], logits_sb[:])
            stats = small.tile([128, 2], f32, tag="st")
            nc.vector.reduce_max(stats[:, 0:1], gmax[:],
                                 axis=mybir.AxisListType.X)
            m_neg = small.tile([128, 1], f32, tag="mn")
            nc.vector.tensor_scalar_mul(m_neg[:], stats[:, 0:1], -1.0)
            eg = small.tile([128, NG], f32, tag="eg")
            nc.scalar.activation(eg[:], gmax[:], AF.Exp, bias=m_neg[:, 0:1])
            nc.vector.tensor_mul(eg[:], eg[:], gsum[:])
            nc.vector.reduce_sum(stats[:, 1:2], eg[:],
                                 axis=mybir.AxisListType.X)
            nc.scalar.dma_start(stats_out[:], stats[:])

    nc.compile()
    return nc


def _prep_inputs(inputs, hidden, tau, emb, w_ih, w_hh, b_ih, b_hh,
                 w_lin, b_lin, theta, mu):
    """Host-side sharding: pad/permute/cast into the device layouts."""
    ix = int(np.asarray(inputs).reshape(-1)[0])

    # w_lin^T shards: [c, p, t, k, m] <- w_lin[c*6400 + t*128 + m, k*128 + p]
    wl_pad = np.zeros((V_PAD, H), dtype=F8)
    wl_pad[:V] = (np.asarray(w_lin, dtype=np.float32) * WL_SCALE).astype(F8)
    A = wl_pad.reshape(N_CORES, T_CORE, 128, KH, 128)       # (c, t, m, k, p)
    WL = np.ascontiguousarray(A.transpose(0, 4, 1, 3, 2))   # (c, p, t, k, m)
    WL = WL.reshape(N_CORES, 128, T_CORE * KH * 128)

    # GRU weights: [p, g, j, k, m] <- w[g*1024 + j*128 + m, k*128 + p]
    def gru_t(w, dt, scale=1.0):
        B = (np.asarray(w, dtype=np.float32) * scale).astype(dt)
        B = B.reshape(3, KH, 128, KH, 128)                  # (g, j, m, k, p)
        return np.ascontiguousarray(B.transpose(4, 0, 1, 3, 2)).reshape(128, -1)

    WI = gru_t(w_ih, F8, WL_SCALE)
    WH = gru_t(w_hh, BF16)

    def gbias(b):
        return np.ascontiguousarray(
            np.asarray(b, dtype=np.float32).reshape(3, KH, 128).transpose(2, 0, 1)
        ).reshape(128, 24)

    BI = gbias(b_ih)
    BH = gbias(b_hh)

    bl_pad = np.full(V_PAD, NEG_BIG, dtype=np.float32)
    bl_pad[:V] = np.asarray(b_lin, dtype=np.float32)
    BL = np.ascontiguousarray(
        bl_pad.reshape(N_CORES, T_CORE, 128).transpose(0, 2, 1))  # (c, p, t)

    x_row = np.asarray(emb[ix], dtype=np.float32)
    X = np.ascontiguousarray(x_row.reshape(KH, 128).T)      # [p, k]
    Hh = np.ascontiguousarray(
        np.asarray(hidden, dtype=np.float32).reshape(KH, 128).T)

    # packed small-input tensor, layout must match _build_program
    PK_W = 2 * KH + 48 + T_CORE + 3
    PK = np.empty((N_CORES, 128, PK_W), dtype=np.float32)
    PK[:, :, 0:KH] = X
    PK[:, :, KH:2 * KH] = Hh
    PK[:, :, 16:40] = BI
    PK[:, :, 40:64] = BH
    PK[:, :, 64:64 + T_CORE] = BL
    PK[:, :, 114] = float(np.asarray(theta)[ix])
    PK[:, :, 115] = float(np.asarray(mu)[ix])
    PK[:, :, 116] = float(np.asarray(tau))

    in_maps = []
    for c in range(N_CORES):
        in_maps.append({
            "wl": WL[c], "wi": WI, "wh": WH, "pk": PK[c],
        })
    return in_maps


def kernel(inputs, hidden, tau, emb, w_ih, w_hh, b_ih, b_hh,
           w_lin, b_lin, theta, mu):
    key = "prog"
    if key not in _PROGRAM_CACHE:
        _PROGRAM_CACHE[key] = _build_program()
    nc = _PROGRAM_CACHE[key]

    in_maps = _prep_inputs(inputs, hidden, tau, emb, w_ih, w_hh, b_ih, b_hh,
                           w_lin, b_lin, theta, mu)

    res = bass_utils.run_bass_kernel_spmd(nc, in_maps,
                                          core_ids=list(range(N_CORES)))

    # ---- host unshard ----
    logits_big = np.concatenate(
        [res.results[c]["logits_out"] for c in range(N_CORES)], axis=1)
    # column order: c*T_CORE + t ; logit[v] with v = (c*T_CORE + t)*128 + p
    logits_full = logits_big.T.reshape(V_PAD)[:V]

    m1 = np.concatenate(
        [res.results[c]["stats_out"][:, 0] for c in range(N_CORES)])
    s1 = np.concatenate(
        [res.results[c]["stats_out"][:, 1] for c in range(N_CORES)])
    m1 = m1.astype(np.float64)
    s1 = s1.astype(np.float64)
    M = m1.max()
    S = float((np.exp(m1 - M) * s1).sum())
    lse = M + np.log(S)

    out = (logits_full - np.float32(lse))[None, :].astype(np.float32)

    h_new = res.results[0]["h_out"].T.reshape(1, 1, H).astype(np.float32)
    return out, h_new
